# revision 9
# baseline (speedup 1.0000x reference)
"""Trainium2 Bass kernel: single-head causal attention (B=8, T=2048, E=1024, H=64).

Sharding: data-parallel over the batch dim — one batch element per NeuronCore,
8 cores, no collectives.

All matmuls in bf16 (tolerance is 2e-2; bf16 keeps rel-err ~1e-3):
  - X is shipped PRE-TRANSPOSED and bf16 from the host ([E, T] layout packed
    per 512-wide t-column), killing the on-chip PE transpose pass, its
    PSUM->SBUF copies, and half the HBM traffic of the f32 variant.
  - Projections per 512-wide column c: QK^T = [Wq|Wk]^T @ XT_c (M=128 chain),
    V'^T = [Wv|0]^T @ XT_c (M=65; the zero 65th row turns into the ones row
    of V' via the +[bv;1] bias fused into the PSUM->SBUF copy).
    K^T re-based to partitions 0:63 of a zero-padded [128, T] tile by DMA
    (full-128 contraction keeps the PE activity monitor at full clock).
    V'^T is PE-transposed back to natural [t, 66-stride] tiles.
  - Scores S^T[k, q-chunk c] = K_j Q^T for causal k-chunks j <= 4c+3, two
    chunks share one 2-bank PSUM tile so full pairs need a single ScalarE
    exp instruction (exp throughput is co-critical with the PE in the score
    phase); causal -1e30 mask added to diagonal 128x128 blocks by DVE.
  - PV accumulates into O' PSUM [65, 512] (row 64 = softmax denominator Z).
  - Next column's projection matmuls are interleaved between score pairs so
    ScalarE's exp stream always trails the PE without stalling it, and the
    PE never idles (p-state stays at 2.4 GHz).
  Output per core: [65, 2048] = [unnormalized O^T; Z]. Host divides by Z and
  transposes during the unshard (part of gather).
"""

import numpy as np
import ml_dtypes

import concourse.bass as bass
import concourse.bacc as bacc
import concourse.mybir as mybir
from concourse.tile import TileContext
from concourse.bass_utils import run_bass_kernel_spmd

T = 2048
E = 1024
H = 64
P = 128
TC = 512  # t/q chunk width (one PSUM bank of f32)
NT = T // P  # 16 t-tiles
NE = E // P  # 8 e-chunks
NTC = T // TC  # 4 t-chunks
NCORES = 8
VS = 68  # v_sb/psv inner stride (>= 66, 8B-aligned in bf16)

F32 = mybir.dt.float32
BF16 = mybir.dt.bfloat16
AF = mybir.ActivationFunctionType
BF16NP = ml_dtypes.bfloat16

# bf16 const block column layout (per partition)
CBH_IDENT = 0  # [128] identity
CBH_WQK = P  # [NE * 2H] = 1024, [e_chunk, m] with m: 0:64=Wq, 64:128=Wk
CBH_WV = CBH_WQK + NE * 2 * H  # [NE * (H+1)] = 520, col H of each chunk = 0
CBH_M01 = CBH_WV + NE * (H + 1)  # [128] causal mask: 1 keep (y>=p), 0 drop
CBH_COLS = CBH_M01 + P
# f32 const block (biases)
CBF_BQK = 0  # [1] bq on partitions 0:64, bk on 64:128
CBF_BV1 = CBF_BQK + 1  # [1] rows 0:64 = bv, row 64 = 1.0 (ones row of V')
CBF_COLS = CBF_BV1 + 1


def pack_consts(Wq, Wk, Wv, bq, bk, bv):
    cbh = np.zeros((P, CBH_COLS), dtype=np.float32)
    cbh[:, CBH_IDENT : CBH_IDENT + P] = np.eye(P, dtype=np.float32)
    wqk = np.zeros((P, NE, 2 * H), dtype=np.float32)
    wqk[:, :, 0:H] = Wq.reshape(NE, P, H).transpose(1, 0, 2)
    wqk[:, :, H : 2 * H] = Wk.reshape(NE, P, H).transpose(1, 0, 2)
    cbh[:, CBH_WQK:CBH_WV] = wqk.reshape(P, NE * 2 * H)
    wv1 = np.zeros((P, NE, H + 1), dtype=np.float32)
    wv1[:, :, 0:H] = Wv.reshape(NE, P, H).transpose(1, 0, 2)
    cbh[:, CBH_WV:CBH_M01] = wv1.reshape(P, NE * (H + 1))
    p_idx = np.arange(P)[:, None]
    y_idx = np.arange(P)[None, :]
    cbh[:, CBH_M01 : CBH_M01 + P] = (y_idx >= p_idx).astype(np.float32)

    cbf = np.zeros((P, CBF_COLS), dtype=np.float32)
    cbf[0:H, CBF_BQK] = bq
    cbf[H : 2 * H, CBF_BQK] = bk
    cbf[0:H, CBF_BV1] = bv
    cbf[H, CBF_BV1] = 1.0
    return cbh.astype(BF16NP), cbf


def pack_x(xi):
    """[T, E] f32 -> [P, NTC, NE, TC] bf16 with X^T chunk (c, ne) contiguous."""
    xt = np.asarray(xi, dtype=np.float32).T.astype(BF16NP)  # [E, T]
    return np.ascontiguousarray(
        xt.reshape(NE, P, NTC, TC).transpose(1, 2, 0, 3)
    )


def build_kernel():
    nc = bacc.Bacc("TRN2", target_bir_lowering=False, debug=False)
    x = nc.dram_tensor("x", [P, NTC, NE, TC], BF16, kind="ExternalInput")
    cbh = nc.dram_tensor("cbh", [P, CBH_COLS], BF16, kind="ExternalInput")
    cbf = nc.dram_tensor("cbf", [P, CBF_COLS], F32, kind="ExternalInput")
    out = nc.dram_tensor("out", [H + 1, T], F32, kind="ExternalOutput")

    scale = 1.0 / np.sqrt(np.float32(H))

    with TileContext(nc) as tc:
        with (
            tc.tile_pool(name="const", bufs=1) as const,
            tc.tile_pool(name="vt", bufs=2) as vtpool,
            tc.tile_pool(name="es", bufs=3) as espool,
            tc.tile_pool(name="ps_prj", bufs=1, space="PSUM") as ps_prj,
            tc.tile_pool(name="ps_s", bufs=2, space="PSUM") as ps_s,
            tc.tile_pool(name="ps_o", bufs=1, space="PSUM") as ps_o,
        ):
            cbh_sb = const.tile([P, CBH_COLS], BF16)
            cbf_sb = const.tile([P, CBF_COLS], F32)
            xt_sb = const.tile([P, NTC, NE, TC], BF16)
            dummy_sb = const.tile([P, 2], BF16)
            # parallel DMA dispatch: sync carries ident/wqk/x0 (QK(0) gates),
            # scalar (also HWDGE) the small wv1/mask/bias blocks, gpsimd x1
            nc.sync.dma_start(cbh_sb[:, 0:P], cbh[:, 0:P])
            nc.sync.dma_start(cbh_sb[:, P:CBH_WV], cbh[:, P:CBH_WV])
            nc.sync.dma_start(xt_sb[:, 0, 0:4], x[:, 0, 0:4])
            nc.sync.dma_start(xt_sb[:, 0, 4:8], x[:, 0, 4:8])
            nc.scalar.dma_start(cbh_sb[:, CBH_WV:CBH_COLS], cbh[:, CBH_WV:CBH_COLS])
            nc.scalar.dma_start(cbf_sb[:, :], cbf[:, :])
            nc.gpsimd.dma_start(xt_sb[:, 1], x[:, 1])

            ident = cbh_sb[:, CBH_IDENT : CBH_IDENT + P]
            wqk_sb = cbh_sb[:, CBH_WQK:CBH_WV].rearrange(
                "p (c m) -> p c m", m=2 * H
            )
            wv1_sb = cbh_sb[:, CBH_WV:CBH_M01].rearrange(
                "p (c m) -> p c m", m=H + 1
            )
            mask01 = cbh_sb[:, CBH_M01 : CBH_M01 + P]
            bqk_t = cbf_sb[:, CBF_BQK : CBF_BQK + 1]
            bv1_t = cbf_sb[0 : H + 1, CBF_BV1 : CBF_BV1 + 1]

            # persistent activations
            qk_sb = const.tile([P, T], BF16)  # rows 0:64 = Q^T, 64:128 = K^T
            kt_sb = const.tile([P, T], BF16)  # K^T re-based, rows 64:128 zero
            v_sb = const.tile([P, NT, VS], BF16)  # V' natural, cols 0:65 live
            o_sb = const.tile([H + 1, T], F32)
            nc.vector.memset(kt_sb[H : 2 * H, :], 0.0)

            # PE p-state warmup while the first x tiles stream in; the dummy
            # exp pulls the ~1.3us ACT table load off the critical path
            warm = ps_s.tile([P, 2, TC], F32, tag="s")
            for i in range(32):
                nc.tensor.matmul(
                    warm[:, 0, 0:P], ident, ident, start=True, stop=True
                )
            nc.scalar.activation(
                dummy_sb[:, 0:2], warm[:, 0, 0:2], AF.Exp, scale=1.0
            )

            # ---------- emission helpers ----------
            prj = {}  # c -> (pqk, pv) PSUM tiles
            vts = {}  # c -> vt' SBUF tile [65, TC]

            def emit_qk_chain_member(c, ec):
                if ec == 0:
                    prj[c] = (
                        ps_prj.tile([P, TC], F32, tag="pqk", name=f"pqk{c}"),
                        ps_prj.tile([H + 1, TC], F32, tag="pv", name=f"pv{c}"),
                    )
                nc.tensor.matmul(
                    prj[c][0][:],
                    wqk_sb[:, ec, :],
                    xt_sb[:, c, ec, :],
                    start=(ec == 0),
                    stop=(ec == NE - 1),
                )

            def emit_v_chain_member(c, ec):
                nc.tensor.matmul(
                    prj[c][1][:],
                    wv1_sb[:, ec, :],
                    xt_sb[:, c, ec, :],
                    start=(ec == 0),
                    stop=(ec == NE - 1),
                )

            def emit_copies(c):
                # PSUM->SBUF with biases fused; K^T re-base via gpsimd DMA
                pqk, pv = prj.pop(c)
                c0 = c * TC
                nc.vector.tensor_scalar_add(qk_sb[:, c0 : c0 + TC], pqk[:], bqk_t)
                nc.gpsimd.dma_start(
                    kt_sb[0:H, c0 : c0 + TC], qk_sb[H : 2 * H, c0 : c0 + TC]
                )
                vt = vtpool.tile([H + 1, TC], BF16, tag="vt", name=f"vt{c}")
                vts[c] = vt
                nc.vector.tensor_scalar_add(vt[:, :], pv[:], bv1_t)

            def emit_vtr(c):
                # V'^T [65, TC] -> natural V' tiles [128, 66] (col 65 = junk 0)
                vt = vts.pop(c)
                psv = ps_prj.tile([P, 4, VS], BF16, tag="psv", name=f"psv{c}")
                for tt in range(4):
                    nc.tensor.transpose(
                        psv[:, tt, 0:66],
                        vt[:, tt * P : (tt + 1) * P],
                        ident[0 : H + 1, 0:66],
                    )
                nc.vector.tensor_copy(
                    v_sb[:, 4 * c : 4 * c + 4, 0:66], psv[:, :, 0:66]
                )

            def chunk_geom(j, c):
                k0 = j * P
                q0 = max(c * TC, k0)
                return k0, q0, (c + 1) * TC - q0

            def emit_scores(j, c, ps):
                k0, q0, w = chunk_geom(j, c)
                nc.tensor.matmul(
                    ps[:, j % 2, 0:w],
                    kt_sb[:, k0 : k0 + P],
                    qk_sb[:, q0 : q0 + w],
                    start=True,
                    stop=True,
                )

            def emit_exp(k, c, ps, es):
                # exp depends only on the score matmuls; the causal mask is
                # applied multiplicatively to es afterwards (diag chunks), off
                # the S->exp critical path (the exp->PV lag absorbs the DVE)
                if 2 * k + 1 < 4 * c:  # full pair: one wide exp
                    nc.scalar.activation(
                        es[:, :, :], ps[:, :, :], AF.Exp, scale=float(scale)
                    )
                    return
                for r in range(2):  # diagonal pair: width-exact per chunk
                    _, _, w = chunk_geom(2 * k + r, c)
                    nc.scalar.activation(
                        es[:, r, 0:w], ps[:, r, 0:w], AF.Exp, scale=float(scale)
                    )
                    nc.vector.tensor_tensor(
                        es[:, r, 0:P], es[:, r, 0:P], mask01,
                        mybir.AluOpType.mult,
                    )

            def emit_pv(k, c, es, o_c):
                njc = 4 * c + 4
                for r in range(2):
                    j = 2 * k + r
                    _, q0, w = chunk_geom(j, c)
                    a = q0 - c * TC
                    nc.tensor.matmul(
                        o_c[:, a : a + w],
                        v_sb[:, j, 0:65],
                        es[:, r, 0:w],
                        start=(j == 0),
                        stop=(j == njc - 1),
                    )

            # ---------- schedule ----------
            # prologue: projections for column 0
            for ec in range(NE):
                emit_qk_chain_member(0, ec)
            for ec in range(NE):
                emit_v_chain_member(0, ec)
            emit_copies(0)

            for c in range(NTC):
                # x prefetch two columns ahead
                if c + 2 < NTC:
                    nc.sync.dma_start(xt_sb[:, c + 2], x[:, c + 2])
                npair = 2 * c + 2
                # members of next column's projection chains to interleave
                members = []
                if c + 1 < NTC:
                    members = [
                        (emit_qk_chain_member, c + 1, ec) for ec in range(NE)
                    ] + [(emit_v_chain_member, c + 1, ec) for ec in range(NE)]
                # interleave members only into the full (non-diagonal) pairs
                # so diag scores+exps of this column aren't pushed late
                nfull = 2 * c
                quota = (
                    max(1, len(members) // nfull) if members and nfull else 0
                )

                o_c = ps_o.tile([H + 1, TC], F32, tag="o", name=f"o{c}")
                pairs = {}
                for k in range(npair):
                    ps = ps_s.tile([P, 2, TC], F32, tag="s", name=f"s{k}_{c}")
                    es = espool.tile([P, 2, TC], BF16, tag="es", name=f"es{k}_{c}")
                    pairs[k] = es
                    emit_scores(2 * k, c, ps)
                    emit_scores(2 * k + 1, c, ps)
                    emit_exp(k, c, ps, es)
                    if k == 1:
                        emit_vtr(c)  # V' tiles ready before first PV
                    if k < nfull:
                        for _ in range(quota):
                            if members:
                                f, a1, a2 = members.pop(0)
                                f(a1, a2)
                    if k >= 2:
                        emit_pv(k - 2, c, pairs.pop(k - 2), o_c)
                while members:
                    f, a1, a2 = members.pop(0)
                    f(a1, a2)
                for k in (npair - 2, npair - 1):
                    emit_pv(k, c, pairs.pop(k), o_c)
                c0 = c * TC
                nc.vector.tensor_copy(o_sb[:, c0 : c0 + TC], o_c[:])
                nc.sync.dma_start(out[:, c0 : c0 + TC], o_sb[:, c0 : c0 + TC])
                if c + 1 < NTC:
                    emit_copies(c + 1)
    nc.compile()
    return nc


_NC_CACHE = None


def _get_nc():
    global _NC_CACHE
    if _NC_CACHE is None:
        _NC_CACHE = build_kernel()
    return _NC_CACHE


def make_in_maps(batch_x, Wk, bk, Wq, bq, Wv, bv):
    cbh, cbf = pack_consts(
        np.asarray(Wq, dtype=np.float32),
        np.asarray(Wk, dtype=np.float32),
        np.asarray(Wv, dtype=np.float32),
        np.asarray(bq, dtype=np.float32),
        np.asarray(bk, dtype=np.float32),
        np.asarray(bv, dtype=np.float32),
    )
    return [
        {"x": pack_x(batch_x[i]), "cbh": cbh, "cbf": cbf}
        for i in range(NCORES)
    ]


def unshard(results):
    outs = []
    for i in range(NCORES):
        o = results[i]["out"]  # [65, 2048]
        outs.append((o[:H] / o[H : H + 1]).T)  # normalize + transpose
    return np.stack(outs).astype(np.float32)


def kernel(batch_x, Wk, bk, Wq, bq, Wv, bv):
    nc = _get_nc()
    in_maps = make_in_maps(batch_x, Wk, bk, Wq, bq, Wv, bv)
    res = run_bass_kernel_spmd(nc, in_maps, list(range(NCORES)))
    return unshard(res.results)


if __name__ == "__main__":
    rng = np.random.default_rng(0)
    inputs = {
        "batch_x": rng.standard_normal((NCORES, T, E), dtype=np.float32),
        "Wk": rng.standard_normal((E, H), dtype=np.float32) * 0.03,
        "bk": rng.standard_normal((H,), dtype=np.float32) * 0.03,
        "Wq": rng.standard_normal((E, H), dtype=np.float32) * 0.03,
        "bq": rng.standard_normal((H,), dtype=np.float32) * 0.03,
        "Wv": rng.standard_normal((E, H), dtype=np.float32) * 0.03,
        "bv": rng.standard_normal((H,), dtype=np.float32) * 0.03,
    }
    out = kernel(**inputs)
    print(out.shape, out.dtype)
